# revision 9
# baseline (speedup 1.0000x reference)
"""GroupQuantLinear on 8 Trainium2 NeuronCores.

y[b,s,o] = x[b,s,:] @ W[o,:] + bias[o], where W is dequantized on-device from
4-bit packed weights with per-(o, group) affine scale/bias (groups of 256 along
the 4096-wide input dim).

Sharding: tensor-parallel on out_features (8 shards of 2048 rows); x replicated.

Per-core kernel (Bass/Tile), single fused phase, W^T resident in SBUF:
  * Packed words are host-transposed to [word, o] uint16 (the int32 words only
    use 16 bits), so each nibble-plane unpack lands directly in a [k', o]
    k-subtile of the SBUF-resident W^T — no on-chip transposes at all.
  * Dequant is 2 DVE ops per k-subtile: a fused scalar_tensor_tensor
    (pk & (0xF<<4p)) * (S * 16^-p)  — the pre-scaled S planes are exact in
    bf16 (power-of-2) — then + B.  S/B group-broadcast planes are host-built.
  * Main matmul streams x^T tiles as the stationary operand (host pre-tiled so
    each m-tile is one contiguous 8 KiB/partition DMA), W^T slices moving,
    fp32 PSUM accumulation over 32 k-subtiles in dequant-production order.
    The first two m-tiles are interleaved over k so the PE has work while the
    dequant window completes; the bias is folded into the PSUM->SBUF eviction.
  * y is written bf16 and widened to f32 on the host.
"""

import numpy as np

B, S, IN, OUT, G = 2, 2048, 4096, 16384, 16
NCORES = 8
OSH = OUT // NCORES       # 2048 out rows per core
BS = B * S                # 4096
NW = IN // 4              # 1024 packed u16 words per out row
P = 128
NKS = IN // P             # 32 k-subtiles
NJ = NW // P              # 8 word-tiles
NPL = 4                   # nibble planes
NMT = BS // P             # 32 m-tiles
NN = OSH // 512           # 4 n-slices
# dequant production order of k-subtiles: wtile-major, plane-minor
KORD = [p * NJ + j for j in range(NJ) for p in range(NPL)]

_COMPILED = {}


def _build_nc():
    from contextlib import ExitStack

    import concourse.mybir as mybir
    import concourse.tile as tile
    from concourse import bacc
    from concourse.bass import ds, ts

    f32 = mybir.dt.float32
    bf16 = mybir.dt.bfloat16
    u16 = mybir.dt.uint16
    AL = mybir.AluOpType

    nc = bacc.Bacc(None, target_bir_lowering=False)

    xt4 = nc.dram_tensor("xt4", [BS, NKS, P], bf16, kind="ExternalInput")
    wpkt = nc.dram_tensor("wpkt", [NW, OSH], u16, kind="ExternalInput")
    sbcp = nc.dram_tensor("sbcp", [NJ, P, OSH], bf16, kind="ExternalInput")
    bbc = nc.dram_tensor("bbc", [NW, OSH], bf16, kind="ExternalInput")
    biasb = nc.dram_tensor("biasb", [P, NN, 512], bf16, kind="ExternalInput")
    y = nc.dram_tensor("y", [BS, NN, 512], bf16, kind="ExternalOutput")

    with tile.TileContext(nc) as tc:
        with ExitStack() as ctx:
            const = ctx.enter_context(tc.tile_pool(name="const", bufs=1))
            wres = ctx.enter_context(tc.tile_pool(name="wres", bufs=1))
            dq_pk = ctx.enter_context(tc.tile_pool(name="dqpk", bufs=2))
            dq_s = ctx.enter_context(tc.tile_pool(name="dqs", bufs=2))
            dq_b = ctx.enter_context(tc.tile_pool(name="dqb", bufs=2))
            dq_q = ctx.enter_context(tc.tile_pool(name="dqq", bufs=1))
            dq_t = ctx.enter_context(tc.tile_pool(name="dqt", bufs=1))
            xp = ctx.enter_context(tc.tile_pool(name="xp", bufs=3))
            yp = ctx.enter_context(tc.tile_pool(name="yp", bufs=4))
            psp = ctx.enter_context(tc.tile_pool(name="psp", bufs=2, space="PSUM"))

            bias_sb = const.tile([P, NN, 512], bf16)
            nc.sync.dma_start(bias_sb[:], biasb[:])

            # W^T resident: [k'-within-subtile (part), k-subtile, o] bf16
            wt = wres.tile([P, NKS, OSH], bf16)

            # ---- dequant: 2 DVE ops per k-subtile, no transposes ----
            for j in range(NJ):
                pk = dq_pk.tile([P, OSH], u16, tag="pk")
                nc.sync.dma_start(pk[:], wpkt[ts(j, P), :])
                sp = dq_s.tile([P, OSH], bf16, tag="sp")
                nc.sync.dma_start(sp[:], sbcp[j])
                bt = dq_b.tile([P, OSH], bf16, tag="bt")
                nc.sync.dma_start(bt[:], bbc[ts(j, P), :])
                for p in range(NPL):
                    ksub = p * NJ + j
                    q = dq_q.tile([P, OSH], u16, tag="q")
                    nc.vector.tensor_scalar(
                        q[:], pk[:], 4 * p, 0xF,
                        AL.logical_shift_right, AL.bitwise_and,
                    )
                    tmp = dq_t.tile([P, OSH], bf16, tag="tmp")
                    nc.vector.tensor_mul(tmp[:], q[:], sp[:])
                    nc.vector.tensor_add(wt[:, ksub, :], tmp[:], bt[:])

            # ---- main matmul: y = x @ W^T + bias ----
            def x_dma(m):
                t = xp.tile([P, NKS, P], bf16, tag="x")
                nc.sync.dma_start(t[:], xt4[ts(m, P), :, :])
                return t

            def evict(m, pst):
                for h in range(2):
                    ys = yp.tile([P, 2, 512], bf16, tag="ys")
                    nc.vector.tensor_add(
                        ys[:], pst[:, ts(h, 2), :], bias_sb[:, ts(h, 2), :]
                    )
                    nc.sync.dma_start(y[ts(m, P), ts(h, 2), :], ys[:])

            # window: first two m-tiles interleaved over k (production order)
            x0, x1 = x_dma(0), x_dma(1)
            ps0 = psp.tile([P, NN, 512], f32, tag="ps")
            ps1 = psp.tile([P, NN, 512], f32, tag="ps")
            for idx, kk in enumerate(KORD):
                for xt_t, pst in ((x0, ps0), (x1, ps1)):
                    for n in range(NN):
                        nc.tensor.matmul(
                            pst[:, n, :], xt_t[:, kk, :], wt[:, kk, ts(n, 512)],
                            start=(idx == 0), stop=(idx == NKS - 1),
                        )
            evict(0, ps0)
            evict(1, ps1)

            # steady state: one m-tile at a time, double-buffered
            for m in range(2, NMT):
                xt_t = x_dma(m)
                pst = psp.tile([P, NN, 512], f32, tag="ps")
                for idx, kk in enumerate(KORD):
                    for n in range(NN):
                        nc.tensor.matmul(
                            pst[:, n, :], xt_t[:, kk, :], wt[:, kk, ts(n, 512)],
                            start=(idx == 0), stop=(idx == NKS - 1),
                        )
                evict(m, pst)

    nc.compile()
    return nc


def _get_compiled():
    if "nc" not in _COMPILED:
        _COMPILED["nc"] = _build_nc()
    return _COMPILED["nc"]


def _marshal(input, w_packed, w_scale, w_bias, bias):
    import ml_dtypes

    BF = ml_dtypes.bfloat16

    x = np.ascontiguousarray(input, dtype=np.float32).reshape(BS, IN)
    # x^T with rows permuted plane-major (k' = plane*NW + word), then tiled so
    # m-tile m, partition pp holds the contiguous 8KiB row xt4[m*P+pp, :, :]
    A = x.T.reshape(NW, NPL, BS).transpose(1, 0, 2).reshape(IN, BS)
    T4 = A.reshape(NKS, P, NMT, P).transpose(2, 1, 0, 3)
    xt4 = np.ascontiguousarray(T4.reshape(BS, NKS, P)).astype(BF)

    in_maps = []
    for c in range(NCORES):
        osl = slice(c * OSH, (c + 1) * OSH)
        wpkt = np.ascontiguousarray(
            w_packed[osl].reshape(OSH, NW).astype(np.uint16).T
        )
        sT = w_scale[osl, :, 0].astype(np.float32).T        # [G, OSH]
        sbcp = np.ascontiguousarray(
            np.repeat(sT, 64, axis=0).reshape(NJ, P, OSH)
        ).astype(BF)
        bbc = np.repeat(
            w_bias[osl, :, 0].astype(np.float32).T, 64, axis=0
        ).astype(BF)
        biasb = np.ascontiguousarray(
            np.broadcast_to(
                bias[osl].astype(np.float32).reshape(1, NN, 512), (P, NN, 512)
            )
        ).astype(BF)
        in_maps.append(
            {
                "xt4": xt4,
                "wpkt": wpkt,
                "sbcp": sbcp,
                "bbc": np.ascontiguousarray(bbc),
                "biasb": biasb,
            }
        )
    return in_maps


def kernel(input, w_packed, w_scale, w_bias, bias, _trace=False, _trace_kwargs=None):
    from concourse.bass_utils import run_bass_kernel_spmd

    nc = _get_compiled()
    in_maps = _marshal(input, w_packed, w_scale, w_bias, bias)
    res = run_bass_kernel_spmd(
        nc,
        in_maps,
        core_ids=list(range(NCORES)),
        trace=_trace,
        **(_trace_kwargs or {}),
    )
    ys = [
        np.asarray(res.results[c]["y"]).reshape(BS, OSH).astype(np.float32)
        for c in range(NCORES)
    ]
    out = np.concatenate(ys, axis=1).reshape(B, S, OUT)
    if _trace:
        return out, res
    return out


# revision 13
# speedup vs baseline: 1.1921x; 1.1921x over previous
"""GroupQuantLinear on 8 Trainium2 NeuronCores.

y[b,s,o] = x[b,s,:] @ W[o,:] + bias[o], where W is dequantized on-device from
4-bit packed weights with per-(o, group) affine scale/bias (groups of 256 along
the 4096-wide input dim).

Sharding: tensor-parallel on out_features (8 shards of 2048 rows); x replicated.

Per-core kernel (Bass/Tile), single fused phase, W^T resident in SBUF:
  * Packed words are host-transposed to [word, o] uint16 (the int32 words only
    use 16 bits), so each nibble-plane unpack lands directly in a [k', o]
    k-subtile of the SBUF-resident W^T — no on-chip transposes at all.
  * Dequant is 2 DVE ops per k-subtile: a fused scalar_tensor_tensor
    (pk & (0xF<<4p)) * (S * 16^-p)  — the pre-scaled S planes are exact in
    bf16 (power-of-2) — then + B.  S/B group-broadcast planes are host-built.
  * Main matmul streams x^T tiles as the stationary operand (host pre-tiled so
    each m-tile is one contiguous 8 KiB/partition DMA), W^T slices moving,
    fp32 PSUM accumulation over 32 k-subtiles in dequant-production order.
    The first two m-tiles are interleaved over k so the PE has work while the
    dequant window completes; the bias is folded into the PSUM->SBUF eviction.
  * y is written bf16 and widened to f32 on the host.
"""

import numpy as np

B, S, IN, OUT, G = 2, 2048, 4096, 16384, 16
NCORES = 8
OSH = OUT // NCORES       # 2048 out rows per core
BS = B * S                # 4096
NW = IN // 4              # 1024 packed u16 words per out row
P = 128
NKS = IN // P             # 32 k-subtiles
NJ = NW // P              # 8 word-tiles
NPL = 4                   # nibble planes
NMT = BS // P             # 32 m-tiles
NN = OSH // 512           # 4 n-slices
# dequant production order of k-subtiles: wtile-major, plane-minor
KORD = [p * NJ + j for j in range(NJ) for p in range(NPL)]

_COMPILED = {}


def _build_nc():
    from contextlib import ExitStack

    import concourse.mybir as mybir
    import concourse.tile as tile
    from concourse import bacc
    from concourse.bass import ds, ts

    f32 = mybir.dt.float32
    bf16 = mybir.dt.bfloat16
    u16 = mybir.dt.uint16
    AL = mybir.AluOpType

    nc = bacc.Bacc(None, target_bir_lowering=False)

    xt4 = nc.dram_tensor("xt4", [BS, NKS, P], bf16, kind="ExternalInput")
    wpkt = nc.dram_tensor("wpkt", [NW, OSH], u16, kind="ExternalInput")
    sbcp = nc.dram_tensor("sbcp", [NJ, P, OSH], bf16, kind="ExternalInput")
    bbc = nc.dram_tensor("bbc", [NW, OSH], bf16, kind="ExternalInput")
    biasb = nc.dram_tensor("biasb", [P, NN, 512], bf16, kind="ExternalInput")
    y = nc.dram_tensor("y", [BS, NN, 512], bf16, kind="ExternalOutput")

    with tile.TileContext(nc) as tc:
        with ExitStack() as ctx:
            const = ctx.enter_context(tc.tile_pool(name="const", bufs=1))
            wres = ctx.enter_context(tc.tile_pool(name="wres", bufs=1))
            dq_pk = ctx.enter_context(tc.tile_pool(name="dqpk", bufs=2))
            dq_s = ctx.enter_context(tc.tile_pool(name="dqs", bufs=2))
            dq_b = ctx.enter_context(tc.tile_pool(name="dqb", bufs=2))
            dq_q = ctx.enter_context(tc.tile_pool(name="dqq", bufs=1))
            dq_t = ctx.enter_context(tc.tile_pool(name="dqt", bufs=1))
            xp = ctx.enter_context(tc.tile_pool(name="xp", bufs=3))
            yp = ctx.enter_context(tc.tile_pool(name="yp", bufs=4))
            psp = ctx.enter_context(tc.tile_pool(name="psp", bufs=2, space="PSUM"))

            # W^T resident: [k'-within-subtile (part), k-subtile, o] bf16
            wt = wres.tile([P, NKS, OSH], bf16)

            def x_dma(m):
                t = xp.tile([P, NKS, P], bf16, tag="x")
                nc.sync.dma_start(t[:], xt4[ts(m, P), :, :])
                return t

            # head-latency: x tiles for the window pair go out first
            x0, x1 = x_dma(0), x_dma(1)
            bias_sb = const.tile([P, NN, 512], bf16)
            nc.sync.dma_start(bias_sb[:], biasb[:])

            # ---- dequant: 2 DVE ops per k-subtile, no transposes ----
            for j in range(NJ):
                pk = dq_pk.tile([P, OSH], u16, tag="pk")
                nc.sync.dma_start(pk[:], wpkt[ts(j, P), :])
                sp = dq_s.tile([P, OSH], bf16, tag="sp")
                nc.sync.dma_start(sp[:], sbcp[j])
                bt = dq_b.tile([P, OSH], bf16, tag="bt")
                nc.sync.dma_start(bt[:], bbc[ts(j, P), :])
                for p in range(NPL):
                    ksub = p * NJ + j
                    q = dq_q.tile([P, OSH], u16, tag="q")
                    nc.vector.tensor_scalar(
                        q[:], pk[:], 4 * p, 0xF,
                        AL.logical_shift_right, AL.bitwise_and,
                    )
                    tmp = dq_t.tile([P, OSH], bf16, tag="tmp")
                    nc.vector.tensor_mul(tmp[:], q[:], sp[:])
                    nc.vector.tensor_add(wt[:, ksub, :], tmp[:], bt[:])

            # ---- main matmul: y = x @ W^T + bias ----
            def evict(m, pst):
                for h in range(2):
                    ys = yp.tile([P, 2, 512], bf16, tag="ys")
                    nc.vector.tensor_add(
                        ys[:], pst[:, ts(h, 2), :], bias_sb[:, ts(h, 2), :]
                    )
                    nc.sync.dma_start(y[ts(m, P), ts(h, 2), :], ys[:])

            # window: first two m-tiles interleaved over k (production order)
            ps0 = psp.tile([P, NN, 512], f32, tag="ps")
            ps1 = psp.tile([P, NN, 512], f32, tag="ps")
            for idx, kk in enumerate(KORD):
                for xt_t, pst in ((x0, ps0), (x1, ps1)):
                    for n in range(NN):
                        nc.tensor.matmul(
                            pst[:, n, :], xt_t[:, kk, :], wt[:, kk, ts(n, 512)],
                            start=(idx == 0), stop=(idx == NKS - 1),
                        )
            evict(0, ps0)
            evict(1, ps1)

            # steady state: one m-tile at a time, double-buffered.
            # k is the INNER loop so the PSUM write address stays fixed for 32
            # consecutive MMs (per-MM bank cycling costs ~45ns/MM).
            for m in range(2, NMT):
                xt_t = x_dma(m)
                pst = psp.tile([P, NN, 512], f32, tag="ps")
                for n in range(NN):
                    for kk in range(NKS):
                        nc.tensor.matmul(
                            pst[:, n, :], xt_t[:, kk, :], wt[:, kk, ts(n, 512)],
                            start=(kk == 0), stop=(kk == NKS - 1),
                        )
                evict(m, pst)

    nc.compile()
    return nc


def _get_compiled():
    if "nc" not in _COMPILED:
        _COMPILED["nc"] = _build_nc()
    return _COMPILED["nc"]


def _marshal(input, w_packed, w_scale, w_bias, bias):
    import ml_dtypes

    BF = ml_dtypes.bfloat16

    x = np.ascontiguousarray(input, dtype=np.float32).reshape(BS, IN)
    # x^T with rows permuted plane-major (k' = plane*NW + word), then tiled so
    # m-tile m, partition pp holds the contiguous 8KiB row xt4[m*P+pp, :, :]
    A = x.T.reshape(NW, NPL, BS).transpose(1, 0, 2).reshape(IN, BS)
    T4 = A.reshape(NKS, P, NMT, P).transpose(2, 1, 0, 3)
    xt4 = np.ascontiguousarray(T4.reshape(BS, NKS, P)).astype(BF)

    in_maps = []
    for c in range(NCORES):
        osl = slice(c * OSH, (c + 1) * OSH)
        wpkt = np.ascontiguousarray(
            w_packed[osl].reshape(OSH, NW).astype(np.uint16).T
        )
        sT = w_scale[osl, :, 0].astype(np.float32).T        # [G, OSH]
        sbcp = np.ascontiguousarray(
            np.repeat(sT, 64, axis=0).reshape(NJ, P, OSH)
        ).astype(BF)
        bbc = np.repeat(
            w_bias[osl, :, 0].astype(np.float32).T, 64, axis=0
        ).astype(BF)
        biasb = np.ascontiguousarray(
            np.broadcast_to(
                bias[osl].astype(np.float32).reshape(1, NN, 512), (P, NN, 512)
            )
        ).astype(BF)
        in_maps.append(
            {
                "xt4": xt4,
                "wpkt": wpkt,
                "sbcp": sbcp,
                "bbc": np.ascontiguousarray(bbc),
                "biasb": biasb,
            }
        )
    return in_maps


def kernel(input, w_packed, w_scale, w_bias, bias, _trace=False, _trace_kwargs=None):
    from concourse.bass_utils import run_bass_kernel_spmd

    nc = _get_compiled()
    in_maps = _marshal(input, w_packed, w_scale, w_bias, bias)
    res = run_bass_kernel_spmd(
        nc,
        in_maps,
        core_ids=list(range(NCORES)),
        trace=_trace,
        **(_trace_kwargs or {}),
    )
    ys = [
        np.asarray(res.results[c]["y"]).reshape(BS, OSH).astype(np.float32)
        for c in range(NCORES)
    ]
    out = np.concatenate(ys, axis=1).reshape(B, S, OUT)
    if _trace:
        return out, res
    return out


# revision 16
# speedup vs baseline: 1.1967x; 1.0039x over previous
"""GroupQuantLinear on 8 Trainium2 NeuronCores.

y[b,s,o] = x[b,s,:] @ W[o,:] + bias[o], where W is dequantized on-device from
4-bit packed weights with per-(o, group) affine scale/bias (groups of 256 along
the 4096-wide input dim).

Sharding: tensor-parallel on out_features (8 shards of 2048 rows); x replicated.

Per-core kernel (Bass/Tile), single fused phase, W^T resident in SBUF:
  * Packed words are host-transposed to [word, o] uint16 (the int32 words only
    use 16 bits), so each nibble-plane unpack lands directly in a [k', o]
    k-subtile of the SBUF-resident W^T — no on-chip transposes at all.
  * Dequant is 2 DVE ops per k-subtile: a fused scalar_tensor_tensor
    (pk & (0xF<<4p)) * (S * 16^-p)  — the pre-scaled S planes are exact in
    bf16 (power-of-2) — then + B.  S/B group-broadcast planes are host-built.
  * Main matmul streams x^T tiles as the stationary operand (host pre-tiled so
    each m-tile is one contiguous 8 KiB/partition DMA), W^T slices moving,
    fp32 PSUM accumulation over 32 k-subtiles in dequant-production order.
    The first two m-tiles are interleaved over k so the PE has work while the
    dequant window completes; the bias is folded into the PSUM->SBUF eviction.
  * y is written bf16 and widened to f32 on the host.
"""

import numpy as np

B, S, IN, OUT, G = 2, 2048, 4096, 16384, 16
NCORES = 8
OSH = OUT // NCORES       # 2048 out rows per core
BS = B * S                # 4096
NW = IN // 4              # 1024 packed u16 words per out row
P = 128
NKS = IN // P             # 32 k-subtiles
NJ = NW // P              # 8 word-tiles
NPL = 4                   # nibble planes
NMT = BS // P             # 32 m-tiles
NN = OSH // 512           # 4 n-slices
# dequant production order of k-subtiles: wtile-major, plane-minor
KORD = [p * NJ + j for j in range(NJ) for p in range(NPL)]

_COMPILED = {}


def _build_nc():
    from contextlib import ExitStack

    import concourse.mybir as mybir
    import concourse.tile as tile
    from concourse import bacc
    from concourse.bass import ds, ts

    f32 = mybir.dt.float32
    bf16 = mybir.dt.bfloat16
    u16 = mybir.dt.uint16
    AL = mybir.AluOpType

    nc = bacc.Bacc(None, target_bir_lowering=False)

    xt4 = nc.dram_tensor("xt4", [BS, NKS, P], bf16, kind="ExternalInput")
    wpkt = nc.dram_tensor("wpkt", [NW, OSH], u16, kind="ExternalInput")
    sbcp = nc.dram_tensor("sbcp", [NJ, P, OSH], bf16, kind="ExternalInput")
    bbc = nc.dram_tensor("bbc", [NW, OSH], bf16, kind="ExternalInput")
    biasb = nc.dram_tensor("biasb", [P, NN, 512], bf16, kind="ExternalInput")
    y = nc.dram_tensor("y", [BS, NN, 512], bf16, kind="ExternalOutput")

    with tile.TileContext(nc) as tc:
        with ExitStack() as ctx:
            const = ctx.enter_context(tc.tile_pool(name="const", bufs=1))
            wres = ctx.enter_context(tc.tile_pool(name="wres", bufs=1))
            dq_pk = ctx.enter_context(tc.tile_pool(name="dqpk", bufs=2))
            dq_s = ctx.enter_context(tc.tile_pool(name="dqs", bufs=2))
            dq_b = ctx.enter_context(tc.tile_pool(name="dqb", bufs=2))
            dq_q = ctx.enter_context(tc.tile_pool(name="dqq", bufs=1))
            dq_t = ctx.enter_context(tc.tile_pool(name="dqt", bufs=1))
            xp = ctx.enter_context(tc.tile_pool(name="xp", bufs=3))
            yp = ctx.enter_context(tc.tile_pool(name="yp", bufs=4))
            psp = ctx.enter_context(tc.tile_pool(name="psp", bufs=2, space="PSUM"))

            # W^T resident: [k'-within-subtile (part), k-subtile, o] bf16
            wt = wres.tile([P, NKS, OSH], bf16)

            def x_dma(m):
                t = xp.tile([P, NKS, P], bf16, tag="x")
                nc.sync.dma_start(t[:], xt4[ts(m, P), :, :])
                return t

            # j=0 dequant DMAs head the serial DVE critical chain — issue first
            def dq_dma(j):
                pk = dq_pk.tile([P, OSH], u16, tag="pk")
                nc.sync.dma_start(pk[:], wpkt[ts(j, P), :])
                sp = dq_s.tile([P, OSH], bf16, tag="sp")
                nc.sync.dma_start(sp[:], sbcp[j])
                bt = dq_b.tile([P, OSH], bf16, tag="bt")
                nc.sync.dma_start(bt[:], bbc[ts(j, P), :])
                return pk, sp, bt

            j0_tiles = dq_dma(0)
            x0, x1 = x_dma(0), x_dma(1)
            bias_sb = const.tile([P, NN, 512], bf16)
            nc.sync.dma_start(bias_sb[:], biasb[:])

            # ---- dequant: 3 DVE ops per k-subtile, no transposes ----
            for j in range(NJ):
                pk, sp, bt = j0_tiles if j == 0 else dq_dma(j)
                for p in range(NPL):
                    ksub = p * NJ + j
                    q = dq_q.tile([P, OSH], u16, tag="q")
                    nc.vector.tensor_scalar(
                        q[:], pk[:], 4 * p, 0xF,
                        AL.logical_shift_right, AL.bitwise_and,
                    )
                    tmp = dq_t.tile([P, OSH], bf16, tag="tmp")
                    nc.vector.tensor_mul(tmp[:], q[:], sp[:])
                    nc.vector.tensor_add(wt[:, ksub, :], tmp[:], bt[:])

            # ---- main matmul: y = x @ W^T + bias ----
            def evict_n(m, n, pst):
                ys = yp.tile([P, 512], bf16, tag="ys")
                nc.vector.tensor_add(ys[:], pst[:, n, :], bias_sb[:, n, :])
                nc.sync.dma_start(y[ts(m, P), n, :], ys[:])

            def evict(m, pst):
                for n in range(NN):
                    evict_n(m, n, pst)

            # window: first two m-tiles interleaved over k (production order)
            ps0 = psp.tile([P, NN, 512], f32, tag="ps")
            ps1 = psp.tile([P, NN, 512], f32, tag="ps")
            for idx, kk in enumerate(KORD):
                for xt_t, pst in ((x0, ps0), (x1, ps1)):
                    for n in range(NN):
                        nc.tensor.matmul(
                            pst[:, n, :], xt_t[:, kk, :], wt[:, kk, ts(n, 512)],
                            start=(idx == 0), stop=(idx == NKS - 1),
                        )
            evict(0, ps0)
            evict(1, ps1)

            # steady state: one m-tile at a time, double-buffered.
            # k is the INNER loop so the PSUM write address stays fixed for 32
            # consecutive MMs (per-MM bank cycling costs ~45ns/MM).
            for m in range(2, NMT):
                xt_t = x_dma(m)
                pst = psp.tile([P, NN, 512], f32, tag="ps")
                for n in range(NN):
                    for kk in range(NKS):
                        nc.tensor.matmul(
                            pst[:, n, :], xt_t[:, kk, :], wt[:, kk, ts(n, 512)],
                            start=(kk == 0), stop=(kk == NKS - 1),
                        )
                    evict_n(m, n, pst)

    nc.compile()
    return nc


def _get_compiled():
    if "nc" not in _COMPILED:
        _COMPILED["nc"] = _build_nc()
    return _COMPILED["nc"]


def _marshal(input, w_packed, w_scale, w_bias, bias):
    import ml_dtypes

    BF = ml_dtypes.bfloat16

    x = np.ascontiguousarray(input, dtype=np.float32).reshape(BS, IN)
    # x^T with rows permuted plane-major (k' = plane*NW + word), then tiled so
    # m-tile m, partition pp holds the contiguous 8KiB row xt4[m*P+pp, :, :]
    A = x.T.reshape(NW, NPL, BS).transpose(1, 0, 2).reshape(IN, BS)
    T4 = A.reshape(NKS, P, NMT, P).transpose(2, 1, 0, 3)
    xt4 = np.ascontiguousarray(T4.reshape(BS, NKS, P)).astype(BF)

    in_maps = []
    for c in range(NCORES):
        osl = slice(c * OSH, (c + 1) * OSH)
        wpkt = np.ascontiguousarray(
            w_packed[osl].reshape(OSH, NW).astype(np.uint16).T
        )
        sT = w_scale[osl, :, 0].astype(np.float32).T        # [G, OSH]
        sbcp = np.ascontiguousarray(
            np.repeat(sT, 64, axis=0).reshape(NJ, P, OSH)
        ).astype(BF)
        bbc = np.repeat(
            w_bias[osl, :, 0].astype(np.float32).T, 64, axis=0
        ).astype(BF)
        biasb = np.ascontiguousarray(
            np.broadcast_to(
                bias[osl].astype(np.float32).reshape(1, NN, 512), (P, NN, 512)
            )
        ).astype(BF)
        in_maps.append(
            {
                "xt4": xt4,
                "wpkt": wpkt,
                "sbcp": sbcp,
                "bbc": np.ascontiguousarray(bbc),
                "biasb": biasb,
            }
        )
    return in_maps


def kernel(input, w_packed, w_scale, w_bias, bias, _trace=False, _trace_kwargs=None):
    from concourse.bass_utils import run_bass_kernel_spmd

    nc = _get_compiled()
    in_maps = _marshal(input, w_packed, w_scale, w_bias, bias)
    res = run_bass_kernel_spmd(
        nc,
        in_maps,
        core_ids=list(range(NCORES)),
        trace=_trace,
        **(_trace_kwargs or {}),
    )
    ys = [
        np.asarray(res.results[c]["y"]).reshape(BS, OSH).astype(np.float32)
        for c in range(NCORES)
    ]
    out = np.concatenate(ys, axis=1).reshape(B, S, OUT)
    if _trace:
        return out, res
    return out


# revision 19
# speedup vs baseline: 1.2166x; 1.0167x over previous
"""GroupQuantLinear on 8 Trainium2 NeuronCores.

y[b,s,o] = x[b,s,:] @ W[o,:] + bias[o], where W is dequantized on-device from
4-bit packed weights with per-(o, group) affine scale/bias (groups of 256 along
the 4096-wide input dim).

Sharding: tensor-parallel on out_features (8 shards of 2048 rows); x replicated.

Per-core kernel (Bass/Tile), single fused phase, W^T resident in SBUF:
  * Packed words are host-transposed to [word, o] uint16 (the int32 words only
    use 16 bits), so each nibble-plane unpack lands directly in a [k', o]
    k-subtile of the SBUF-resident W^T — no on-chip transposes at all.
  * Dequant is 2 DVE ops per k-subtile: a fused scalar_tensor_tensor
    (pk & (0xF<<4p)) * (S * 16^-p)  — the pre-scaled S planes are exact in
    bf16 (power-of-2) — then + B.  S/B group-broadcast planes are host-built.
  * Main matmul streams x^T tiles as the stationary operand (host pre-tiled so
    each m-tile is one contiguous 8 KiB/partition DMA), W^T slices moving,
    fp32 PSUM accumulation over 32 k-subtiles in dequant-production order.
    The first two m-tiles are interleaved over k so the PE has work while the
    dequant window completes; the bias is folded into the PSUM->SBUF eviction.
  * y is written bf16 and widened to f32 on the host.
"""

import numpy as np

B, S, IN, OUT, G = 2, 2048, 4096, 16384, 16
NCORES = 8
OSH = OUT // NCORES       # 2048 out rows per core
BS = B * S                # 4096
NW = IN // 4              # 1024 packed u16 words per out row
P = 128
NKS = IN // P             # 32 k-subtiles
NJ = NW // P              # 8 word-tiles
NPL = 4                   # nibble planes
NMT = BS // P             # 32 m-tiles
NN = OSH // 512           # 4 n-slices
# dequant production order of k-subtiles: wtile-major, plane-minor
KORD = [p * NJ + j for j in range(NJ) for p in range(NPL)]

_COMPILED = {}


def _build_nc():
    from contextlib import ExitStack

    import concourse.mybir as mybir
    import concourse.tile as tile
    from concourse import bacc
    from concourse.bass import ds, ts

    f32 = mybir.dt.float32
    bf16 = mybir.dt.bfloat16
    u16 = mybir.dt.uint16
    AL = mybir.AluOpType

    nc = bacc.Bacc(None, target_bir_lowering=False)

    xt4 = nc.dram_tensor("xt4", [BS, NKS, P], bf16, kind="ExternalInput")
    wpkt = nc.dram_tensor("wpkt", [NW, OSH], u16, kind="ExternalInput")
    sbcp = nc.dram_tensor("sbcp", [NJ, P, OSH], bf16, kind="ExternalInput")
    bbc = nc.dram_tensor("bbc", [NW, OSH], bf16, kind="ExternalInput")
    biasb = nc.dram_tensor("biasb", [P, NN, 512], bf16, kind="ExternalInput")
    y = nc.dram_tensor("y", [BS, NN, 512], bf16, kind="ExternalOutput")

    with tile.TileContext(nc) as tc:
        with ExitStack() as ctx:
            const = ctx.enter_context(tc.tile_pool(name="const", bufs=1))
            wres = ctx.enter_context(tc.tile_pool(name="wres", bufs=1))
            dq_pk = ctx.enter_context(tc.tile_pool(name="dqpk", bufs=2))
            dq_s = ctx.enter_context(tc.tile_pool(name="dqs", bufs=2))
            dq_b = ctx.enter_context(tc.tile_pool(name="dqb", bufs=2))
            dq_q = ctx.enter_context(tc.tile_pool(name="dqq", bufs=1))
            dq_t = ctx.enter_context(tc.tile_pool(name="dqt", bufs=1))
            xp = ctx.enter_context(tc.tile_pool(name="xp", bufs=4))
            yp = ctx.enter_context(tc.tile_pool(name="yp", bufs=3))
            stp = ctx.enter_context(tc.tile_pool(name="stp", bufs=2))
            psp = ctx.enter_context(tc.tile_pool(name="psp", bufs=2, space="PSUM"))

            # W^T resident: [k'-within-subtile (part), k-subtile, o] bf16
            wt = wres.tile([P, NKS, OSH], bf16)

            def x_dma(m):
                t = xp.tile([P, NKS, P], bf16, tag="x")
                nc.sync.dma_start(t[:], xt4[ts(m, P), :, :])
                return t

            # j=0 dequant DMAs head the serial DVE critical chain — issue first
            def dq_dma(j):
                pk = dq_pk.tile([P, OSH], u16, tag="pk")
                nc.sync.dma_start(pk[:], wpkt[ts(j, P), :])
                sp = dq_s.tile([P, OSH], bf16, tag="sp")
                nc.sync.dma_start(sp[:], sbcp[j])
                bt = dq_b.tile([P, OSH], bf16, tag="bt")
                nc.sync.dma_start(bt[:], bbc[ts(j, P), :])
                return pk, sp, bt

            j0_tiles = dq_dma(0)
            x0, x1 = x_dma(0), x_dma(1)
            bias_sb = const.tile([P, NN, 512], bf16)
            nc.sync.dma_start(bias_sb[:], biasb[:])

            # ---- dequant: 3 DVE ops per k-subtile, no transposes ----
            for j in range(NJ):
                pk, sp, bt = j0_tiles if j == 0 else dq_dma(j)
                for p in range(NPL):
                    ksub = p * NJ + j
                    q = dq_q.tile([P, OSH], u16, tag="q")
                    nc.vector.tensor_scalar(
                        q[:], pk[:], 4 * p, 0xF,
                        AL.logical_shift_right, AL.bitwise_and,
                    )
                    tmp = dq_t.tile([P, OSH], bf16, tag="tmp")
                    nc.vector.tensor_mul(tmp[:], q[:], sp[:])
                    nc.vector.tensor_add(wt[:, ksub, :], tmp[:], bt[:])

            # ---- main matmul: y = x @ W^T + bias ----
            def evict_n(m, n, pst):
                ys = yp.tile([P, 512], bf16, tag="ys")
                nc.vector.tensor_add(ys[:], pst[:, n, :], bias_sb[:, n, :])
                nc.sync.dma_start(y[ts(m, P), n, :], ys[:])

            def evict(m, pst):
                for n in range(NN):
                    evict_n(m, n, pst)

            # ACT warm-up: prime the activation table before the window needs it
            warm = yp.tile([P, 512], bf16, tag="ys")
            nc.scalar.copy(warm[:, 0:8], bias_sb[:, 0, 0:8])

            # ---- dequant-window schedule ----
            # Any m-tile that accumulates all 32 k-subtiles can only evict when
            # dequant ends, so plain scheduling caps PE work in the window at 2
            # PSUM slots (~55us) while dequant takes ~105us.  Instead:
            #   A: m0/m1 accumulate only KC0 (wtiles 0-3, interleaved, paced by
            #      production); ACT parks the half-k partials in SBUF (bf16).
            #   B: m2/m3 walk the full KORD interleaved, paced by wtiles 4-7.
            #   C: m0/m1 re-run KC1 at full speed, merge partial + psum + bias.
            KC0, KC1 = KORD[:16], KORD[16:]

            def mm_pair(pair, korder, start, stop):
                for idx, kk in enumerate(korder):
                    for xt_t, pst in pair:
                        for n in range(NN):
                            nc.tensor.matmul(
                                pst[:, n, :], xt_t[:, kk, :], wt[:, kk, ts(n, 512)],
                                start=(start and idx == 0),
                                stop=(stop and idx == len(korder) - 1),
                            )

            # phase A
            psA = [psp.tile([P, NN, 512], f32, tag="ps", name=f"psA{i}") for i in range(2)]
            mm_pair(((x0, psA[0]), (x1, psA[1])), KC0, True, True)
            parts = []
            for t in range(2):
                stg = stp.tile([P, NN, 512], bf16, tag="part")
                nc.scalar.copy(stg[:], psA[t][:])
                parts.append(stg)

            # phase B
            x2, x3 = x_dma(2), x_dma(3)
            psB = [psp.tile([P, NN, 512], f32, tag="ps", name=f"psB{i}") for i in range(2)]
            mm_pair(((x2, psB[0]), (x3, psB[1])), KORD, True, True)
            evict(2, psB[0])
            evict(3, psB[1])

            # phase C: m0/m1 second half + merge
            for t in range(2):
                xt_t = x_dma(t)
                pst = psp.tile([P, NN, 512], f32, tag="ps")
                for idx, kk in enumerate(KC1):
                    for n in range(NN):
                        nc.tensor.matmul(
                            pst[:, n, :], xt_t[:, kk, :], wt[:, kk, ts(n, 512)],
                            start=(idx == 0), stop=(idx == len(KC1) - 1),
                        )
                for n in range(NN):
                    tmp = yp.tile([P, 512], bf16, tag="ys")
                    nc.vector.tensor_add(tmp[:], pst[:, n, :], parts[t][:, n, :])
                    ys = yp.tile([P, 512], bf16, tag="ys")
                    nc.vector.tensor_add(ys[:], tmp[:], bias_sb[:, n, :])
                    nc.sync.dma_start(y[ts(t, P), n, :], ys[:])

            # steady state: one m-tile at a time, double-buffered.
            # k is the INNER loop so the PSUM write address stays fixed for 32
            # consecutive MMs (per-MM bank cycling costs ~45ns/MM).
            for m in range(4, NMT):
                xt_t = x_dma(m)
                pst = psp.tile([P, NN, 512], f32, tag="ps")
                for n in range(NN):
                    for kk in range(NKS):
                        nc.tensor.matmul(
                            pst[:, n, :], xt_t[:, kk, :], wt[:, kk, ts(n, 512)],
                            start=(kk == 0), stop=(kk == NKS - 1),
                        )
                    evict_n(m, n, pst)

    nc.compile()
    return nc


def _get_compiled():
    if "nc" not in _COMPILED:
        _COMPILED["nc"] = _build_nc()
    return _COMPILED["nc"]


def _marshal(input, w_packed, w_scale, w_bias, bias):
    import ml_dtypes

    BF = ml_dtypes.bfloat16

    x = np.ascontiguousarray(input, dtype=np.float32).reshape(BS, IN)
    # x^T with rows permuted plane-major (k' = plane*NW + word), then tiled so
    # m-tile m, partition pp holds the contiguous 8KiB row xt4[m*P+pp, :, :]
    A = x.T.reshape(NW, NPL, BS).transpose(1, 0, 2).reshape(IN, BS)
    T4 = A.reshape(NKS, P, NMT, P).transpose(2, 1, 0, 3)
    xt4 = np.ascontiguousarray(T4.reshape(BS, NKS, P)).astype(BF)

    in_maps = []
    for c in range(NCORES):
        osl = slice(c * OSH, (c + 1) * OSH)
        wpkt = np.ascontiguousarray(
            w_packed[osl].reshape(OSH, NW).astype(np.uint16).T
        )
        sT = w_scale[osl, :, 0].astype(np.float32).T        # [G, OSH]
        sbcp = np.ascontiguousarray(
            np.repeat(sT, 64, axis=0).reshape(NJ, P, OSH)
        ).astype(BF)
        bbc = np.repeat(
            w_bias[osl, :, 0].astype(np.float32).T, 64, axis=0
        ).astype(BF)
        biasb = np.ascontiguousarray(
            np.broadcast_to(
                bias[osl].astype(np.float32).reshape(1, NN, 512), (P, NN, 512)
            )
        ).astype(BF)
        in_maps.append(
            {
                "xt4": xt4,
                "wpkt": wpkt,
                "sbcp": sbcp,
                "bbc": np.ascontiguousarray(bbc),
                "biasb": biasb,
            }
        )
    return in_maps


def kernel(input, w_packed, w_scale, w_bias, bias, _trace=False, _trace_kwargs=None):
    from concourse.bass_utils import run_bass_kernel_spmd

    nc = _get_compiled()
    in_maps = _marshal(input, w_packed, w_scale, w_bias, bias)
    res = run_bass_kernel_spmd(
        nc,
        in_maps,
        core_ids=list(range(NCORES)),
        trace=_trace,
        **(_trace_kwargs or {}),
    )
    ys = [
        np.asarray(res.results[c]["y"]).reshape(BS, OSH).astype(np.float32)
        for c in range(NCORES)
    ]
    out = np.concatenate(ys, axis=1).reshape(B, S, OUT)
    if _trace:
        return out, res
    return out


# revision 21
# speedup vs baseline: 1.2281x; 1.0095x over previous
"""GroupQuantLinear on 8 Trainium2 NeuronCores.

y[b,s,o] = x[b,s,:] @ W[o,:] + bias[o], where W is dequantized on-device from
4-bit packed weights with per-(o, group) affine scale/bias (groups of 256 along
the 4096-wide input dim).

Sharding: tensor-parallel on out_features (8 shards of 2048 rows); x replicated.

Per-core kernel (Bass/Tile), single fused phase, W^T resident in SBUF:
  * Packed words are host-transposed to [word, o] uint16 (the int32 words only
    use 16 bits), so each nibble-plane unpack lands directly in a [k', o]
    k-subtile of the SBUF-resident W^T — no on-chip transposes at all.
  * Dequant is 2 DVE ops per k-subtile: a fused scalar_tensor_tensor
    (pk & (0xF<<4p)) * (S * 16^-p)  — the pre-scaled S planes are exact in
    bf16 (power-of-2) — then + B.  S/B group-broadcast planes are host-built.
  * Main matmul streams x^T tiles as the stationary operand (host pre-tiled so
    each m-tile is one contiguous 8 KiB/partition DMA), W^T slices moving,
    fp32 PSUM accumulation over 32 k-subtiles in dequant-production order.
    The first two m-tiles are interleaved over k so the PE has work while the
    dequant window completes; the bias is folded into the PSUM->SBUF eviction.
  * y is written bf16 and widened to f32 on the host.
"""

import numpy as np

B, S, IN, OUT, G = 2, 2048, 4096, 16384, 16
NCORES = 8
OSH = OUT // NCORES       # 2048 out rows per core
BS = B * S                # 4096
NW = IN // 4              # 1024 packed u16 words per out row
P = 128
NKS = IN // P             # 32 k-subtiles
NJ = NW // P              # 8 word-tiles
NPL = 4                   # nibble planes
NMT = BS // P             # 32 m-tiles
NN = OSH // 512           # 4 n-slices
# dequant production order of k-subtiles: wtile-major, plane-minor
KORD = [p * NJ + j for j in range(NJ) for p in range(NPL)]

_COMPILED = {}


def _build_nc():
    from contextlib import ExitStack

    import concourse.mybir as mybir
    import concourse.tile as tile
    from concourse import bacc
    from concourse.bass import ds, ts

    f32 = mybir.dt.float32
    bf16 = mybir.dt.bfloat16
    u16 = mybir.dt.uint16
    AL = mybir.AluOpType

    nc = bacc.Bacc(None, target_bir_lowering=False)

    xt4 = nc.dram_tensor("xt4", [BS, NKS, P], bf16, kind="ExternalInput")
    wpkt = nc.dram_tensor("wpkt", [NW, OSH], u16, kind="ExternalInput")
    sbcp = nc.dram_tensor("sbcp", [NJ, P, OSH], bf16, kind="ExternalInput")
    bbc = nc.dram_tensor("bbc", [NW, OSH], bf16, kind="ExternalInput")
    biasb = nc.dram_tensor("biasb", [P, NN, 512], bf16, kind="ExternalInput")
    y = nc.dram_tensor("y", [BS, NN, 512], bf16, kind="ExternalOutput")

    with tile.TileContext(nc) as tc:
        with ExitStack() as ctx:
            const = ctx.enter_context(tc.tile_pool(name="const", bufs=1))
            wres = ctx.enter_context(tc.tile_pool(name="wres", bufs=1))
            dq_pk = ctx.enter_context(tc.tile_pool(name="dqpk", bufs=2))
            dq_s = ctx.enter_context(tc.tile_pool(name="dqs", bufs=2))
            dq_b = ctx.enter_context(tc.tile_pool(name="dqb", bufs=2))
            dq_q = ctx.enter_context(tc.tile_pool(name="dqq", bufs=1))
            dq_t = ctx.enter_context(tc.tile_pool(name="dqt", bufs=1))
            xp = ctx.enter_context(tc.tile_pool(name="xp", bufs=4))
            yp = ctx.enter_context(tc.tile_pool(name="yp", bufs=3))
            stp = ctx.enter_context(tc.tile_pool(name="stp", bufs=2))
            scr = ctx.enter_context(tc.tile_pool(name="scr", bufs=1, space="DRAM"))
            psp = ctx.enter_context(tc.tile_pool(name="psp", bufs=2, space="PSUM"))

            # W^T resident: [k'-within-subtile (part), k-subtile, o] bf16
            wt = wres.tile([P, NKS, OSH], bf16)

            def x_dma(m):
                t = xp.tile([P, NKS, P], bf16, tag="x")
                nc.sync.dma_start(t[:], xt4[ts(m, P), :, :])
                return t

            # j=0 dequant DMAs head the serial DVE critical chain — issue first
            def dq_dma(j):
                pk = dq_pk.tile([P, OSH], u16, tag="pk")
                nc.sync.dma_start(pk[:], wpkt[ts(j, P), :])
                sp = dq_s.tile([P, OSH], bf16, tag="sp")
                nc.sync.dma_start(sp[:], sbcp[j])
                bt = dq_b.tile([P, OSH], bf16, tag="bt")
                nc.sync.dma_start(bt[:], bbc[ts(j, P), :])
                return pk, sp, bt

            j0_tiles = dq_dma(0)
            x0, x1 = x_dma(0), x_dma(1)
            bias_sb = const.tile([P, NN, 512], bf16)
            nc.sync.dma_start(bias_sb[:], biasb[:])

            # ---- dequant: 3 DVE ops per k-subtile, no transposes ----
            for j in range(NJ):
                pk, sp, bt = j0_tiles if j == 0 else dq_dma(j)
                for p in range(NPL):
                    ksub = p * NJ + j
                    q = dq_q.tile([P, OSH], u16, tag="q")
                    nc.vector.tensor_scalar(
                        q[:], pk[:], 4 * p, 0xF,
                        AL.logical_shift_right, AL.bitwise_and,
                    )
                    tmp = dq_t.tile([P, OSH], bf16, tag="tmp")
                    nc.vector.tensor_mul(tmp[:], q[:], sp[:])
                    nc.vector.tensor_add(wt[:, ksub, :], tmp[:], bt[:])

            # ---- main matmul: y = x @ W^T + bias ----
            def evict_n(m, n, pst):
                ys = yp.tile([P, 512], bf16, tag="ys")
                nc.vector.tensor_add(ys[:], pst[:, n, :], bias_sb[:, n, :])
                nc.sync.dma_start(y[ts(m, P), n, :], ys[:])

            def evict(m, pst):
                for n in range(NN):
                    evict_n(m, n, pst)

            # ACT warm-up: prime the activation table before the window needs it
            warm = yp.tile([P, 512], bf16, tag="ys")
            nc.scalar.copy(warm[:, 0:8], bias_sb[:, 0, 0:8])

            # ---- dequant-window schedule ----
            # Any m-tile that accumulates all 32 k-subtiles can only evict when
            # dequant ends; PSUM caps concurrent accumulation at 8 banks, so
            # PE work in the window is bank-turnover-bound.  Chunk the first
            # six m-tiles so the banks recycle as production advances:
            #   A1: m0/m1 over wtiles {0,1}; partials parked via ACT -> DRAM.
            #   A2: m2/m3 over wtiles {0..3}; partials parked in SBUF.
            #   B : m4/m5 over the full KORD, paced by wtiles 4-7.
            #   C': m2/m3 over wtiles {4..7}, merge SBUF partial + bias.
            #   C : m0/m1 over wtiles {2..7}, merge reloaded DRAM partial.
            def mm_pair(pair, korder, start, stop):
                for idx, kk in enumerate(korder):
                    for xt_t, pst in pair:
                        for n in range(NN):
                            nc.tensor.matmul(
                                pst[:, n, :], xt_t[:, kk, :], wt[:, kk, ts(n, 512)],
                                start=(start and idx == 0),
                                stop=(stop and idx == len(korder) - 1),
                            )

            def merge_evict(m, pst, part):
                for n in range(NN):
                    tmp = yp.tile([P, 512], bf16, tag="ys")
                    nc.vector.tensor_add(tmp[:], pst[:, n, :], part[:, n, :])
                    ys = yp.tile([P, 512], bf16, tag="ys")
                    nc.vector.tensor_add(ys[:], tmp[:], bias_sb[:, n, :])
                    nc.sync.dma_start(y[ts(m, P), n, :], ys[:])

            # phase A1: m0/m1 over wtiles {0,1}; partials to DRAM scratch
            psA = [psp.tile([P, NN, 512], f32, tag="ps", name=f"psA{i}") for i in range(2)]
            mm_pair(((x0, psA[0]), (x1, psA[1])), KORD[:8], True, True)
            pdram = []
            for t in range(2):
                stg = stp.tile([P, NN, 512], bf16, tag="part")
                nc.scalar.copy(stg[:], psA[t][:])
                pd = scr.tile([P, NN, 512], bf16, name=f"pdram{t}")
                nc.sync.dma_start(pd[:], stg[:])
                pdram.append(pd)

            # phase A2: m2/m3 over wtiles {0..3}; partials stay in SBUF
            x2, x3 = x_dma(2), x_dma(3)
            psA2 = [psp.tile([P, NN, 512], f32, tag="ps", name=f"psA2{i}") for i in range(2)]
            mm_pair(((x2, psA2[0]), (x3, psA2[1])), KORD[:16], True, True)
            parts23 = []
            for t in range(2):
                stg = stp.tile([P, NN, 512], bf16, tag="part")
                nc.scalar.copy(stg[:], psA2[t][:])
                parts23.append(stg)

            # phase B: m4/m5 over the full production order
            x4, x5 = x_dma(4), x_dma(5)
            psB = [psp.tile([P, NN, 512], f32, tag="ps", name=f"psB{i}") for i in range(2)]
            mm_pair(((x4, psB[0]), (x5, psB[1])), KORD, True, True)
            evict(4, psB[0])
            evict(5, psB[1])

            # phase C': m2/m3 over wtiles {4..7}, merge SBUF partials
            for t in range(2):
                xt_t = x_dma(2 + t)
                pst = psp.tile([P, NN, 512], f32, tag="ps")
                for idx, kk in enumerate(KORD[16:]):
                    for n in range(NN):
                        nc.tensor.matmul(
                            pst[:, n, :], xt_t[:, kk, :], wt[:, kk, ts(n, 512)],
                            start=(idx == 0), stop=(idx == 15),
                        )
                merge_evict(2 + t, pst, parts23[t])

            # phase C: m0/m1 over wtiles {2..7}, merge reloaded DRAM partials
            for t in range(2):
                xt_t = x_dma(t)
                pr = stp.tile([P, NN, 512], bf16, tag="part")
                nc.sync.dma_start(pr[:], pdram[t][:])
                pst = psp.tile([P, NN, 512], f32, tag="ps")
                for idx, kk in enumerate(KORD[8:]):
                    for n in range(NN):
                        nc.tensor.matmul(
                            pst[:, n, :], xt_t[:, kk, :], wt[:, kk, ts(n, 512)],
                            start=(idx == 0), stop=(idx == 23),
                        )
                merge_evict(t, pst, pr)

            # steady state: one m-tile at a time, double-buffered.
            # k is the INNER loop so the PSUM write address stays fixed for 32
            # consecutive MMs (per-MM bank cycling costs ~45ns/MM).
            for m in range(6, NMT):
                xt_t = x_dma(m)
                pst = psp.tile([P, NN, 512], f32, tag="ps")
                for n in range(NN):
                    for kk in range(NKS):
                        nc.tensor.matmul(
                            pst[:, n, :], xt_t[:, kk, :], wt[:, kk, ts(n, 512)],
                            start=(kk == 0), stop=(kk == NKS - 1),
                        )
                    evict_n(m, n, pst)

    nc.compile()
    return nc


def _get_compiled():
    if "nc" not in _COMPILED:
        _COMPILED["nc"] = _build_nc()
    return _COMPILED["nc"]


def _marshal(input, w_packed, w_scale, w_bias, bias):
    import ml_dtypes

    BF = ml_dtypes.bfloat16

    x = np.ascontiguousarray(input, dtype=np.float32).reshape(BS, IN)
    # x^T with rows permuted plane-major (k' = plane*NW + word), then tiled so
    # m-tile m, partition pp holds the contiguous 8KiB row xt4[m*P+pp, :, :]
    A = x.T.reshape(NW, NPL, BS).transpose(1, 0, 2).reshape(IN, BS)
    T4 = A.reshape(NKS, P, NMT, P).transpose(2, 1, 0, 3)
    xt4 = np.ascontiguousarray(T4.reshape(BS, NKS, P)).astype(BF)

    in_maps = []
    for c in range(NCORES):
        osl = slice(c * OSH, (c + 1) * OSH)
        wpkt = np.ascontiguousarray(
            w_packed[osl].reshape(OSH, NW).astype(np.uint16).T
        )
        sT = w_scale[osl, :, 0].astype(np.float32).T        # [G, OSH]
        sbcp = np.ascontiguousarray(
            np.repeat(sT, 64, axis=0).reshape(NJ, P, OSH)
        ).astype(BF)
        bbc = np.repeat(
            w_bias[osl, :, 0].astype(np.float32).T, 64, axis=0
        ).astype(BF)
        biasb = np.ascontiguousarray(
            np.broadcast_to(
                bias[osl].astype(np.float32).reshape(1, NN, 512), (P, NN, 512)
            )
        ).astype(BF)
        in_maps.append(
            {
                "xt4": xt4,
                "wpkt": wpkt,
                "sbcp": sbcp,
                "bbc": np.ascontiguousarray(bbc),
                "biasb": biasb,
            }
        )
    return in_maps


def kernel(input, w_packed, w_scale, w_bias, bias, _trace=False, _trace_kwargs=None):
    from concourse.bass_utils import run_bass_kernel_spmd

    nc = _get_compiled()
    in_maps = _marshal(input, w_packed, w_scale, w_bias, bias)
    res = run_bass_kernel_spmd(
        nc,
        in_maps,
        core_ids=list(range(NCORES)),
        trace=_trace,
        **(_trace_kwargs or {}),
    )
    ys = [
        np.asarray(res.results[c]["y"]).reshape(BS, OSH).astype(np.float32)
        for c in range(NCORES)
    ]
    out = np.concatenate(ys, axis=1).reshape(B, S, OUT)
    if _trace:
        return out, res
    return out


# revision 27
# speedup vs baseline: 1.2389x; 1.0088x over previous
"""GroupQuantLinear on 8 Trainium2 NeuronCores.

y[b,s,o] = x[b,s,:] @ W[o,:] + bias[o], where W is dequantized on-device from
4-bit packed weights with per-(o, group) affine scale/bias (groups of 256 along
the 4096-wide input dim).

Sharding: tensor-parallel on out_features (8 shards of 2048 rows); x replicated.

Per-core kernel (Bass/Tile), single fused phase, W^T resident in SBUF:
  * Packed words are host-transposed to [word, o] uint16 (the int32 words only
    use 16 bits), so each nibble-plane unpack lands directly in a [k', o]
    k-subtile of the SBUF-resident W^T — no on-chip transposes at all.
  * Dequant is 2 DVE ops per k-subtile: a fused scalar_tensor_tensor
    (pk & (0xF<<4p)) * (S * 16^-p)  — the pre-scaled S planes are exact in
    bf16 (power-of-2) — then + B.  S/B group-broadcast planes are host-built.
  * Main matmul streams x^T tiles as the stationary operand (host pre-tiled so
    each m-tile is one contiguous 8 KiB/partition DMA), W^T slices moving,
    fp32 PSUM accumulation over 32 k-subtiles in dequant-production order.
    The first two m-tiles are interleaved over k so the PE has work while the
    dequant window completes; the bias is folded into the PSUM->SBUF eviction.
  * y is written bf16 and widened to f32 on the host.
"""

import numpy as np

B, S, IN, OUT, G = 2, 2048, 4096, 16384, 16
NCORES = 8
OSH = OUT // NCORES       # 2048 out rows per core
BS = B * S                # 4096
NW = IN // 4              # 1024 packed u16 words per out row
P = 128
NKS = IN // P             # 32 k-subtiles
NJ = NW // P              # 8 word-tiles
NPL = 4                   # nibble planes
NMT = BS // P             # 32 m-tiles
NN = OSH // 512           # 4 n-slices
# dequant production order of k-subtiles: wtile-major, plane-minor
KORD = [p * NJ + j for j in range(NJ) for p in range(NPL)]

_COMPILED = {}


def _build_nc():
    from contextlib import ExitStack

    import concourse.mybir as mybir
    import concourse.tile as tile
    from concourse import bacc
    from concourse.bass import ds, ts

    f32 = mybir.dt.float32
    bf16 = mybir.dt.bfloat16
    u16 = mybir.dt.uint16
    AL = mybir.AluOpType

    nc = bacc.Bacc(None, target_bir_lowering=False)

    xt4 = nc.dram_tensor("xt4", [BS, NKS, P], bf16, kind="ExternalInput")
    wpkt = nc.dram_tensor("wpkt", [NW, OSH], u16, kind="ExternalInput")
    sbcp = nc.dram_tensor("sbcp", [NJ, P, OSH], bf16, kind="ExternalInput")
    bbc = nc.dram_tensor("bbc", [NW, OSH], bf16, kind="ExternalInput")
    biasb = nc.dram_tensor("biasb", [P, NN, 512], bf16, kind="ExternalInput")
    y = nc.dram_tensor("y", [BS, NN, 512], bf16, kind="ExternalOutput")

    with tile.TileContext(nc) as tc:
        with ExitStack() as ctx:
            const = ctx.enter_context(tc.tile_pool(name="const", bufs=1))
            wres = ctx.enter_context(tc.tile_pool(name="wres", bufs=1))
            dq_pk = ctx.enter_context(tc.tile_pool(name="dqpk", bufs=2))
            dq_s = ctx.enter_context(tc.tile_pool(name="dqs", bufs=2))
            dq_b = ctx.enter_context(tc.tile_pool(name="dqb", bufs=2))
            dq_q = ctx.enter_context(tc.tile_pool(name="dqq", bufs=1))
            dq_t = ctx.enter_context(tc.tile_pool(name="dqt", bufs=1))
            xp = ctx.enter_context(tc.tile_pool(name="xp", bufs=4))
            yp = ctx.enter_context(tc.tile_pool(name="yp", bufs=3))
            stp = ctx.enter_context(tc.tile_pool(name="stp", bufs=2))
            scr = ctx.enter_context(tc.tile_pool(name="scr", bufs=1, space="DRAM"))
            psp = ctx.enter_context(tc.tile_pool(name="psp", bufs=2, space="PSUM"))

            # W^T resident: [k'-within-subtile (part), k-subtile, o] bf16
            wt = wres.tile([P, NKS, OSH], bf16)

            def x_dma(m):
                t = xp.tile([P, NKS, P], bf16, tag="x")
                nc.sync.dma_start(t[:], xt4[ts(m, P), :, :])
                return t

            # j=0 dequant DMAs head the serial DVE critical chain — issue first
            def dq_dma(j):
                pk = dq_pk.tile([P, OSH], u16, tag="pk")
                nc.sync.dma_start(pk[:], wpkt[ts(j, P), :])
                sp = dq_s.tile([P, OSH], bf16, tag="sp")
                nc.sync.dma_start(sp[:], sbcp[j])
                bt = dq_b.tile([P, OSH], bf16, tag="bt")
                nc.sync.dma_start(bt[:], bbc[ts(j, P), :])
                return pk, sp, bt

            j0_tiles = dq_dma(0)
            x0, x1 = x_dma(0), x_dma(1)
            bias_sb = const.tile([P, NN, 512], bf16)
            nc.sync.dma_start(bias_sb[:], biasb[:])

            # ---- dequant: 3 DVE ops per k-subtile, no transposes ----
            for j in range(NJ):
                pk, sp, bt = j0_tiles if j == 0 else dq_dma(j)
                for p in range(NPL):
                    ksub = p * NJ + j
                    q = dq_q.tile([P, OSH], u16, tag="q")
                    nc.vector.tensor_scalar(
                        q[:], pk[:], 4 * p, 0xF,
                        AL.logical_shift_right, AL.bitwise_and,
                    )
                    tmp = dq_t.tile([P, OSH], bf16, tag="tmp")
                    nc.vector.tensor_mul(tmp[:], q[:], sp[:])
                    nc.vector.tensor_add(wt[:, ksub, :], tmp[:], bt[:])

            # ---- main matmul: y = x @ W^T + bias ----
            def evict_n(m, n, pst):
                ys = yp.tile([P, 512], bf16, tag="ys")
                nc.vector.tensor_add(ys[:], pst[:, n, :], bias_sb[:, n, :])
                nc.sync.dma_start(y[ts(m, P), n, :], ys[:])

            def evict(m, pst):
                for n in range(NN):
                    evict_n(m, n, pst)

            # ACT warm-up: prime the activation table before the window needs it
            warm = yp.tile([P, 512], bf16, tag="ys")
            nc.scalar.copy(warm[:, 0:8], bias_sb[:, 0, 0:8])

            # ---- dequant-window schedule ----
            # Any m-tile that accumulates all 32 k-subtiles can only evict when
            # dequant ends; PSUM caps concurrent accumulation at 8 banks, so
            # PE work in the window is bank-turnover-bound.  Chunk the first
            # six m-tiles so the banks recycle as production advances:
            #   A1: m0/m1 over wtiles {0,1}; partials parked via ACT -> DRAM.
            #   A2: m2/m3 over wtiles {0..3}; partials parked in SBUF.
            #   B : m4/m5 over the full KORD, paced by wtiles 4-7.
            #   C': m2/m3 over wtiles {4..7}, merge SBUF partial + bias.
            #   C : m0/m1 over wtiles {2..7}, merge reloaded DRAM partial.
            def mm_pair(pair, korder, start, stop):
                for idx, kk in enumerate(korder):
                    for xt_t, pst in pair:
                        for n in range(NN):
                            nc.tensor.matmul(
                                pst[:, n, :], xt_t[:, kk, :], wt[:, kk, ts(n, 512)],
                                start=(start and idx == 0),
                                stop=(stop and idx == len(korder) - 1),
                            )

            def merge_evict(m, pst, part):
                for n in range(NN):
                    tmp = yp.tile([P, 512], bf16, tag="ys")
                    nc.vector.tensor_add(tmp[:], pst[:, n, :], part[:, n, :])
                    ys = yp.tile([P, 512], bf16, tag="ys")
                    nc.vector.tensor_add(ys[:], tmp[:], bias_sb[:, n, :])
                    nc.sync.dma_start(y[ts(m, P), n, :], ys[:])

            # phase A1: m0/m1 over wtiles {0,1}; partials to DRAM scratch
            psA = [psp.tile([P, NN, 512], f32, tag="ps", name=f"psA{i}") for i in range(2)]
            mm_pair(((x0, psA[0]), (x1, psA[1])), KORD[:8], True, True)
            pdram = []
            for t in range(2):
                stg = stp.tile([P, NN, 512], bf16, tag="part")
                nc.scalar.copy(stg[:], psA[t][:])
                pd = scr.tile([P, NN, 512], bf16, name=f"pdram{t}")
                nc.sync.dma_start(pd[:], stg[:])
                pdram.append(pd)

            # phase A2: m2/m3 over wtiles {0..3}; partials also parked to DRAM
            x2, x3 = x_dma(2), x_dma(3)
            psA2 = [psp.tile([P, NN, 512], f32, tag="ps", name=f"psA2{i}") for i in range(2)]
            mm_pair(((x2, psA2[0]), (x3, psA2[1])), KORD[:16], True, True)
            pdram23 = []
            for t in range(2):
                stg = stp.tile([P, NN, 512], bf16, tag="part")
                nc.scalar.copy(stg[:], psA2[t][:])
                pd = scr.tile([P, NN, 512], bf16, name=f"pdram2{t}")
                nc.sync.dma_start(pd[:], stg[:])
                pdram23.append(pd)

            # phase B: m4/m5 over the full production order
            x4, x5 = x_dma(4), x_dma(5)
            psB = [psp.tile([P, NN, 512], f32, tag="ps", name=f"psB{i}") for i in range(2)]
            mm_pair(((x4, psB[0]), (x5, psB[1])), KORD, True, True)
            # evict via ACT so the PSUM slots free immediately — at this point
            # the DVE is still draining the dequant tail
            evB = []
            for t in range(2):
                ev = stp.tile([P, NN, 512], bf16, tag="part")
                nc.scalar.copy(ev[:], psB[t][:])
                evB.append(ev)
            for t in range(2):
                for n in range(NN):
                    ys = yp.tile([P, 512], bf16, tag="ys")
                    nc.vector.tensor_add(ys[:], evB[t][:, n, :], bias_sb[:, n, :])
                    nc.sync.dma_start(y[ts(4 + t, P), n, :], ys[:])

            # phase C': m2/m3 over wtiles {4..7}, merge reloaded partials
            for t in range(2):
                xt_t = x_dma(2 + t)
                pr = stp.tile([P, NN, 512], bf16, tag="part")
                nc.sync.dma_start(pr[:], pdram23[t][:])
                pst = psp.tile([P, NN, 512], f32, tag="ps")
                for idx, kk in enumerate(KORD[16:]):
                    for n in range(NN):
                        nc.tensor.matmul(
                            pst[:, n, :], xt_t[:, kk, :], wt[:, kk, ts(n, 512)],
                            start=(idx == 0), stop=(idx == 15),
                        )
                merge_evict(2 + t, pst, pr)

            # phase C: m0/m1 over wtiles {2..7}, merge reloaded DRAM partials
            for t in range(2):
                xt_t = x_dma(t)
                pr = stp.tile([P, NN, 512], bf16, tag="part")
                nc.sync.dma_start(pr[:], pdram[t][:])
                pst = psp.tile([P, NN, 512], f32, tag="ps")
                for idx, kk in enumerate(KORD[8:]):
                    for n in range(NN):
                        nc.tensor.matmul(
                            pst[:, n, :], xt_t[:, kk, :], wt[:, kk, ts(n, 512)],
                            start=(idx == 0), stop=(idx == 23),
                        )
                merge_evict(t, pst, pr)

            # steady state: one m-tile at a time, double-buffered.
            # k is the INNER loop so the PSUM write address stays fixed for 32
            # consecutive MMs (per-MM bank cycling costs ~45ns/MM).
            for m in range(6, NMT):
                xt_t = x_dma(m)
                pst = psp.tile([P, NN, 512], f32, tag="ps")
                for n in range(NN):
                    for kk in range(NKS):
                        nc.tensor.matmul(
                            pst[:, n, :], xt_t[:, kk, :], wt[:, kk, ts(n, 512)],
                            start=(kk == 0), stop=(kk == NKS - 1),
                        )
                    evict_n(m, n, pst)

    nc.compile()
    return nc


def _get_compiled():
    if "nc" not in _COMPILED:
        _COMPILED["nc"] = _build_nc()
    return _COMPILED["nc"]


def _marshal(input, w_packed, w_scale, w_bias, bias):
    import ml_dtypes

    BF = ml_dtypes.bfloat16

    x = np.ascontiguousarray(input, dtype=np.float32).reshape(BS, IN)
    # x^T with rows permuted plane-major (k' = plane*NW + word), then tiled so
    # m-tile m, partition pp holds the contiguous 8KiB row xt4[m*P+pp, :, :]
    A = x.T.reshape(NW, NPL, BS).transpose(1, 0, 2).reshape(IN, BS)
    T4 = A.reshape(NKS, P, NMT, P).transpose(2, 1, 0, 3)
    xt4 = np.ascontiguousarray(T4.reshape(BS, NKS, P)).astype(BF)

    in_maps = []
    for c in range(NCORES):
        osl = slice(c * OSH, (c + 1) * OSH)
        wpkt = np.ascontiguousarray(
            w_packed[osl].reshape(OSH, NW).astype(np.uint16).T
        )
        sT = w_scale[osl, :, 0].astype(np.float32).T        # [G, OSH]
        sbcp = np.ascontiguousarray(
            np.repeat(sT, 64, axis=0).reshape(NJ, P, OSH)
        ).astype(BF)
        bbc = np.repeat(
            w_bias[osl, :, 0].astype(np.float32).T, 64, axis=0
        ).astype(BF)
        biasb = np.ascontiguousarray(
            np.broadcast_to(
                bias[osl].astype(np.float32).reshape(1, NN, 512), (P, NN, 512)
            )
        ).astype(BF)
        in_maps.append(
            {
                "xt4": xt4,
                "wpkt": wpkt,
                "sbcp": sbcp,
                "bbc": np.ascontiguousarray(bbc),
                "biasb": biasb,
            }
        )
    return in_maps


def kernel(input, w_packed, w_scale, w_bias, bias, _trace=False, _trace_kwargs=None):
    from concourse.bass_utils import run_bass_kernel_spmd

    nc = _get_compiled()
    in_maps = _marshal(input, w_packed, w_scale, w_bias, bias)
    res = run_bass_kernel_spmd(
        nc,
        in_maps,
        core_ids=list(range(NCORES)),
        trace=_trace,
        **(_trace_kwargs or {}),
    )
    ys = [
        np.asarray(res.results[c]["y"]).reshape(BS, OSH).astype(np.float32)
        for c in range(NCORES)
    ]
    out = np.concatenate(ys, axis=1).reshape(B, S, OUT)
    if _trace:
        return out, res
    return out
